# revision 3
# baseline (speedup 1.0000x reference)
"""Trainium2 Bass kernel for nn_DWNBlock (LRU scan + Lipschitz sandwich MLP).

Self-contained: host-side parameter folding (Cayley transforms, scan
constants) in numpy float64, then a fully-unrolled Tile/Bass SPMD program
on 8 NeuronCores, data-parallel over the batch dimension (one batch
element per core).

Device algorithm per core (x^T in channel-major [128, 8192]):
  1. Bu^(t-major) via PE matmuls with x^T slices as the stationary operand
  2. chunked linear-recurrence scan (L=512): pre-scale by lam^-s (DVE),
     shared upper-triangular-ones matmul over in-chunk time (PE),
     sequential cross-chunk carry (tiny DVE column ops), post-scale by
     lam^t with the carry folded in via per-partition-scalar ops (DVE)
  3. y^T = C_re@H_re - C_im@H_im + D@x^T (PE, PSUM-accumulated)
  4. folded MLP: relu(G1) -> relu(G2) -> relu(G3) -> G4, + residual
All matmuls run in float32r (TF32-class, full PE rate).
"""
import math
import os
import sys

for _p in ('/opt/trn_rl_repo',):
    if _p not in sys.path:
        sys.path.insert(0, _p)

import numpy as np

D = 128          # d_model
N = 128          # d_state
H = 512          # MLP hidden
T = 8192         # sequence length
B = 8            # batch
L = 512          # scan chunk length
NCORES = 8
SCALE = 1.0
SQRT2 = math.sqrt(2.0)


# ---------------------------------------------------------------- host prep

def _cayley64(W):
    cout, cin = W.shape
    if cin > cout:
        return _cayley64(W.T).T
    U, V = W[:cin], W[cin:]
    I = np.eye(cin, dtype=W.dtype)
    A = U - U.T + V.T @ V
    iIpA = np.linalg.inv(I + A)
    return np.concatenate([iIpA @ (I - A), -2.0 * V @ iIpA], axis=0)


def _host_prep(p):
    """Fold all parameters into device constants (float64 -> float32)."""
    f8 = np.float64
    nu_log = p['nu_log'].astype(f8)
    theta_log = p['theta_log'].astype(f8)
    gamma_log = p['gamma_log'].astype(f8)
    lam = np.exp(-np.exp(nu_log)) * np.exp(1j * np.exp(theta_log))   # [N]
    Beff = np.exp(gamma_log)[:, None] * (p['B_re'].astype(f8) + 1j * p['B_im'].astype(f8))
    beff_w = np.concatenate([Beff.real.T, Beff.imag.T], axis=1)      # [D, 2N]

    C = p['C_re'].astype(f8) + 1j * p['C_im'].astype(f8)             # [D, N]
    ytw = np.concatenate([C.real.T, (-C.imag).T, p['Dmat'].astype(f8).T], axis=1)  # [128, 384]

    s = np.arange(L)
    loglam = np.log(lam)
    pneg = np.exp(-s[:, None] * loglam[None, :])    # [L, N] = lam^-s
    ppos = np.exp(s[None, :] * loglam[:, None])     # [N, L] = lam^t'
    lamL = lam ** L
    lamL1 = lam ** (L - 1)

    # pneg packed for the wide pre-scale over [t-tile pair, re|im|re|im]:
    # pair q covers s-tiles (2q, 2q+1) of the chunk (q in {0,1}).
    def _pk(j):   # s-tile j of pneg, [128, N]
        return pneg[j * 128:(j + 1) * 128, :]
    pneg_pack = np.concatenate([
        # variant A: s-tiles 0,1     [P0r|P0i|P1r|P1i]
        _pk(0).real, _pk(0).imag, _pk(1).real, _pk(1).imag,
        # variant A swapped          [P0i|P0r|P1i|P1r]
        _pk(0).imag, _pk(0).real, _pk(1).imag, _pk(1).real,
        # variant B: s-tiles 2,3
        _pk(2).real, _pk(2).imag, _pk(3).real, _pk(3).imag,
        # variant B swapped
        _pk(2).imag, _pk(2).real, _pk(3).imag, _pk(3).real,
    ], axis=1)                                       # [128, 4096]

    ppos_pack = np.concatenate([ppos.real, ppos.imag, -ppos.imag], axis=1)  # [128, 1536]

    # tri_ones[s, u] = 1 if s <= u, over [128, 512]; block j of the in-chunk
    # triangular matmul uses tri_ones[:, 0:512-128*j] against psum[:, 128*j:512].
    tri_ones = (np.arange(128)[:, None] <= np.arange(512)[None, :]).astype(f8)

    # carry-chain per-partition scalar columns
    lamcols = np.stack([
        lam.real, lam.imag, -lam.imag,
        lamL.real, lamL.imag, -lamL.imag,
        lamL1.real, lamL1.imag, -lamL1.imag,
    ], axis=1)                                       # [128, 9]

    def _q(Wkey, akey, fout):
        Wd = p[Wkey].astype(f8)
        Q = _cayley64((float(p[akey][0]) / np.linalg.norm(Wd)) * Wd)
        return Q[:, fout:], Q[:, :fout]

    Q1in, Q1out = _q('W1', 'alpha1', H)
    Q2in, Q2out = _q('W2', 'alpha2', H)
    Q3in, Q3out = _q('W3', 'alpha3', H)
    Qlin = _cayley64((float(p['alphal'][0]) / np.linalg.norm(p['Wl'].astype(f8)))
                     * p['Wl'].astype(f8))[:, D:]    # [128, 512]

    e = np.exp
    ps1, ps2, ps3 = p['psi1'].astype(f8), p['psi2'].astype(f8), p['psi3'].astype(f8)
    G1 = SCALE * SCALE * SQRT2 * (Q1in.T * e(-ps1)[None, :])                    # [128, 512]
    G2 = 2.0 * SCALE * (e(ps1)[:, None] * Q1out) @ (Q2in.T * e(-ps2)[None, :])  # [512, 512]
    G3 = 2.0 * SCALE * (e(ps2)[:, None] * Q2out) @ (Q3in.T * e(-ps3)[None, :])  # [512, 512]
    G4 = SQRT2 * SCALE * (e(ps3)[:, None] * Q3out) @ Qlin.T                     # [512, 128]

    out = dict(beff_w=beff_w, ytw=ytw, pneg_pack=pneg_pack, ppos_pack=ppos_pack,
               tri_ones=tri_ones, lamcols=lamcols, g1=G1, g2=G2, g3=G3, g4=G4)
    return {k: np.ascontiguousarray(v, dtype=np.float32) for k, v in out.items()}


# ---------------------------------------------------------------- device program

def _build_program(t_len):
    from concourse import bacc
    import concourse.mybir as mybir
    from concourse.tile import TileContext

    f32 = mybir.dt.float32
    f32r = mybir.dt.float32r
    AL = mybir.AluOpType
    ACT = mybir.ActivationFunctionType
    nchunk = t_len // L

    nc = bacc.Bacc("TRN2", target_bir_lowering=False, debug=False)

    xt_d = nc.dram_tensor("xt", [128, t_len], f32r, kind="ExternalInput").ap()
    beff_d = nc.dram_tensor("beff_w", [128, 256], f32r, kind="ExternalInput").ap()
    ytw_d = nc.dram_tensor("ytw", [128, 384], f32r, kind="ExternalInput").ap()
    pneg_d = nc.dram_tensor("pneg_pack", [128, 2048], f32, kind="ExternalInput").ap()
    ppos_d = nc.dram_tensor("ppos_pack", [128, 1536], f32, kind="ExternalInput").ap()
    tri_d = nc.dram_tensor("tri_ones", [128, 512], f32r, kind="ExternalInput").ap()
    lamc_d = nc.dram_tensor("lamcols", [128, 9], f32, kind="ExternalInput").ap()
    g1_d = nc.dram_tensor("g1", [128, 512], f32r, kind="ExternalInput").ap()
    g2_d = nc.dram_tensor("g2", [512, 512], f32r, kind="ExternalInput").ap()
    g3_d = nc.dram_tensor("g3", [512, 512], f32r, kind="ExternalInput").ap()
    g4_d = nc.dram_tensor("g4", [512, 128], f32r, kind="ExternalInput").ap()
    out_d = nc.dram_tensor("outT", [128, t_len], f32, kind="ExternalOutput").ap()

    def r(ap):
        return ap.bitcast(f32r)

    with TileContext(nc) as tc:
        with (
            tc.tile_pool(name="const", bufs=1) as cpool,
            tc.tile_pool(name="epool", bufs=2) as epool,
            tc.tile_pool(name="upool", bufs=4) as upool,
            tc.tile_pool(name="hpool", bufs=3) as hpool,
            tc.tile_pool(name="tpool", bufs=2) as tpool,
            tc.tile_pool(name="ypool", bufs=3) as ypool,
            tc.tile_pool(name="zpool", bufs=2) as zpool,
            tc.tile_pool(name="opool", bufs=3) as opool,
            tc.tile_pool(name="carry", bufs=3) as carry_pool,
            tc.tile_pool(name="bups", bufs=2, space="PSUM") as bu_ps,
            tc.tile_pool(name="mps", bufs=3, space="PSUM") as m_ps,
            tc.tile_pool(name="mlpps", bufs=3, space="PSUM") as mlp_ps,
        ):
            # ---- constants into SBUF
            xt = cpool.tile([128, t_len], f32r, tag="xt")
            for q in range(max(1, t_len // 2048)):
                w = min(2048, t_len)
                nc.sync.dma_start(xt[:, q * w:(q + 1) * w], xt_d[:, q * w:(q + 1) * w])
            beff = cpool.tile([128, 256], f32r, tag="beff")
            nc.sync.dma_start(beff[:], beff_d[:])
            ytw = cpool.tile([128, 384], f32r, tag="ytw")
            nc.sync.dma_start(ytw[:], ytw_d[:])
            pneg = cpool.tile([128, 2048], f32, tag="pneg")
            nc.sync.dma_start(pneg[:], pneg_d[:])
            ppos = cpool.tile([128, 1536], f32, tag="ppos")
            nc.sync.dma_start(ppos[:], ppos_d[:])
            tri = cpool.tile([128, 512], f32r, tag="tri")
            nc.sync.dma_start(tri[:], tri_d[:])
            lamc = cpool.tile([128, 9], f32, tag="lamc")
            nc.sync.dma_start(lamc[:], lamc_d[:])
            g1 = cpool.tile([128, 512], f32r, tag="g1")
            nc.sync.dma_start(g1[:], g1_d[:])
            g2 = cpool.tile([128, 2048], f32r, tag="g2")
            for k in range(4):
                nc.sync.dma_start(g2[:, k * 512:(k + 1) * 512], g2_d[k * 128:(k + 1) * 128, :])
            g3 = cpool.tile([128, 2048], f32r, tag="g3")
            for k in range(4):
                nc.sync.dma_start(g3[:, k * 512:(k + 1) * 512], g3_d[k * 128:(k + 1) * 128, :])
            g4 = cpool.tile([128, 512], f32r, tag="g4")
            for k in range(4):
                nc.sync.dma_start(g4[:, k * 128:(k + 1) * 128], g4_d[k * 128:(k + 1) * 128, :])
            bias0 = cpool.tile([128, 1], f32, tag="bias0")
            nc.vector.memset(bias0[:], 0.0)

            # lamcols views
            lam_re, lam_im, lam_imn = lamc[:, 0:1], lamc[:, 1:2], lamc[:, 2:3]
            lamL_re, lamL_im, lamL_imn = lamc[:, 3:4], lamc[:, 4:5], lamc[:, 5:6]
            lamL1_re, lamL1_im, lamL1_imn = lamc[:, 6:7], lamc[:, 7:8], lamc[:, 8:9]

            # carry state h_0 = 0
            h_re = carry_pool.tile([128, 1], f32, tag="hre")
            h_im = carry_pool.tile([128, 1], f32, tag="him")
            nc.vector.memset(h_re[:], 0.0)
            nc.vector.memset(h_im[:], 0.0)

            ppos_re = ppos[:, 0:512]
            ppos_im = ppos[:, 512:1024]
            ppos_imn = ppos[:, 1024:1536]

            for c in range(nchunk):
                t0 = c * L
                # ---- Bu + pre-scale: 2 pairs of t-tiles
                upair = []
                for q in range(2):
                    bu = bu_ps.tile([128, 512], f32, tag="bu")
                    for half in range(2):
                        i = 2 * q + half          # s-tile index in chunk
                        lhs = xt[:, t0 + i * 128: t0 + (i + 1) * 128]
                        nc.tensor.matmul(bu[:, half * 256:(half + 1) * 256],
                                         r(lhs), r(beff[:]), start=True, stop=True)
                    e1 = epool.tile([128, 512], f32, tag="e1")
                    e2 = epool.tile([128, 512], f32, tag="e2")
                    pv = pneg[:, q * 1024: q * 1024 + 512]
                    pv_sw = pneg[:, q * 1024 + 512: q * 1024 + 1024]
                    nc.vector.tensor_tensor(e1[:], bu[:], pv, AL.mult)
                    nc.vector.tensor_tensor(e2[:], bu[:], pv_sw, AL.mult)
                    up = upool.tile([128, 512], f32r, tag="upair")
                    # re parts (DVE), im parts (GPSIMD, SBUF-only)
                    for half in range(2):
                        o = half * 256
                        nc.vector.tensor_tensor(up[:, o:o + 128], e1[:, o:o + 128],
                                                e1[:, o + 128:o + 256], AL.subtract)
                        nc.gpsimd.tensor_tensor(up[:, o + 128:o + 256], e2[:, o:o + 128],
                                                e2[:, o + 128:o + 256], AL.add)
                    upair.append(up)

                # ---- in-chunk triangular scan matmuls
                m_re = m_ps.tile([128, 512], f32, tag="m")
                m_im = m_ps.tile([128, 512], f32, tag="m")
                for j in range(4):
                    up = upair[j // 2]
                    o = (j % 2) * 256
                    width = 512 - 128 * j
                    nc.tensor.matmul(m_re[:, 128 * j:512], r(up[:, o:o + 128]),
                                     r(tri[:, 0:width]), start=(j == 0), stop=(j == 3))
                for j in range(4):
                    up = upair[j // 2]
                    o = (j % 2) * 256 + 128
                    width = 512 - 128 * j
                    nc.tensor.matmul(m_im[:, 128 * j:512], r(up[:, o:o + 128]),
                                     r(tri[:, 0:width]), start=(j == 0), stop=(j == 3))

                # ---- carry fold values a = lam * h   (tiny column ops)
                a_re = carry_pool.tile([128, 1], f32, tag="are")
                a_im = carry_pool.tile([128, 1], f32, tag="aim")
                tmp1 = carry_pool.tile([128, 1], f32, tag="ctmp1")
                tmp2 = carry_pool.tile([128, 1], f32, tag="ctmp2")
                nc.vector.tensor_tensor(tmp1[:], h_re[:], lam_re, AL.mult)
                nc.vector.scalar_tensor_tensor(a_re[:], h_im[:], lam_imn, tmp1[:],
                                               AL.mult, AL.add)
                nc.vector.tensor_tensor(tmp2[:], h_re[:], lam_im, AL.mult)
                nc.vector.scalar_tensor_tensor(a_im[:], h_im[:], lam_re, tmp2[:],
                                               AL.mult, AL.add)

                # ---- next carry h' = lamL*h + lamL1*M[:, L-1]
                mr_col = m_re[:, L - 1:L]
                mi_col = m_im[:, L - 1:L]
                c1 = carry_pool.tile([128, 1], f32, tag="c1")
                c2 = carry_pool.tile([128, 1], f32, tag="c2")
                c3 = carry_pool.tile([128, 1], f32, tag="c3")
                h_re_n = carry_pool.tile([128, 1], f32, tag="hre")
                d1 = carry_pool.tile([128, 1], f32, tag="d1")
                d2 = carry_pool.tile([128, 1], f32, tag="d2")
                d3 = carry_pool.tile([128, 1], f32, tag="d3")
                h_im_n = carry_pool.tile([128, 1], f32, tag="him")
                nc.vector.tensor_tensor(c1[:], h_re[:], lamL_re, AL.mult)
                nc.vector.scalar_tensor_tensor(c2[:], h_im[:], lamL_imn, c1[:], AL.mult, AL.add)
                nc.vector.scalar_tensor_tensor(c3[:], mr_col, lamL1_re, c2[:], AL.mult, AL.add)
                nc.vector.scalar_tensor_tensor(h_re_n[:], mi_col, lamL1_imn, c3[:], AL.mult, AL.add)
                nc.vector.tensor_tensor(d1[:], h_im[:], lamL_re, AL.mult)
                nc.vector.scalar_tensor_tensor(d2[:], h_re[:], lamL_im, d1[:], AL.mult, AL.add)
                nc.vector.scalar_tensor_tensor(d3[:], mi_col, lamL1_re, d2[:], AL.mult, AL.add)
                nc.vector.scalar_tensor_tensor(h_im_n[:], mr_col, lamL1_im, d3[:], AL.mult, AL.add)

                # ---- post-scale: H = ppos * (M + bcast(a))
                t1 = tpool.tile([128, 512], f32, tag="t1")
                t2 = tpool.tile([128, 512], f32, tag="t2")
                t3 = tpool.tile([128, 512], f32, tag="t3")
                t4 = tpool.tile([128, 512], f32, tag="t4")
                hre_t = hpool.tile([128, 512], f32r, tag="Hre")
                him_t = hpool.tile([128, 512], f32r, tag="Him")
                nc.vector.scalar_tensor_tensor(t1[:], m_re[:], a_re[:], ppos_re, AL.add, AL.mult)
                nc.vector.scalar_tensor_tensor(t2[:], m_im[:], a_im[:], ppos_imn, AL.add, AL.mult)
                nc.gpsimd.tensor_tensor(hre_t[:], t1[:], t2[:], AL.add)
                nc.vector.scalar_tensor_tensor(t3[:], m_re[:], a_re[:], ppos_im, AL.add, AL.mult)
                nc.vector.scalar_tensor_tensor(t4[:], m_im[:], a_im[:], ppos_re, AL.add, AL.mult)
                nc.gpsimd.tensor_tensor(him_t[:], t3[:], t4[:], AL.add)

                h_re, h_im = h_re_n, h_im_n

                # ---- y^T = Cre@Hre - Cim@Him + D@xT
                y_ps = mlp_ps.tile([128, 512], f32, tag="mlp")
                nc.tensor.matmul(y_ps[:], r(ytw[:, 0:128]), r(hre_t[:]), start=True, stop=False)
                nc.tensor.matmul(y_ps[:], r(ytw[:, 128:256]), r(him_t[:]), start=False, stop=False)
                nc.tensor.matmul(y_ps[:], r(ytw[:, 256:384]), r(xt[:, t0:t0 + 512]),
                                 start=False, stop=True)
                y_sb = ypool.tile([128, 512], f32r, tag="ysb")
                nc.scalar.copy(y_sb[:], y_ps[:])

                # ---- MLP
                z1 = []
                for m in range(4):
                    zp = mlp_ps.tile([128, 512], f32, tag="mlp")
                    nc.tensor.matmul(zp[:], r(g1[:, m * 128:(m + 1) * 128]), r(y_sb[:]),
                                     start=True, stop=True)
                    zt = zpool.tile([128, 512], f32r, tag=f"z1_{m}")
                    nc.scalar.activation(zt[:], zp[:], ACT.Relu, bias=bias0[:])
                    z1.append(zt)
                z2 = []
                for m in range(4):
                    zp = mlp_ps.tile([128, 512], f32, tag="mlp")
                    for k in range(4):
                        nc.tensor.matmul(zp[:], r(g2[:, k * 512 + m * 128: k * 512 + (m + 1) * 128]),
                                         r(z1[k][:]), start=(k == 0), stop=(k == 3))
                    zt = zpool.tile([128, 512], f32r, tag=f"z2_{m}")
                    nc.scalar.activation(zt[:], zp[:], ACT.Relu, bias=bias0[:])
                    z2.append(zt)
                z3 = []
                for m in range(4):
                    zp = mlp_ps.tile([128, 512], f32, tag="mlp")
                    for k in range(4):
                        nc.tensor.matmul(zp[:], r(g3[:, k * 512 + m * 128: k * 512 + (m + 1) * 128]),
                                         r(z2[k][:]), start=(k == 0), stop=(k == 3))
                    zt = zpool.tile([128, 512], f32r, tag=f"z3_{m}")
                    nc.scalar.activation(zt[:], zp[:], ACT.Relu, bias=bias0[:])
                    z3.append(zt)
                zp = mlp_ps.tile([128, 512], f32, tag="mlp")
                for k in range(4):
                    nc.tensor.matmul(zp[:], r(g4[:, k * 128:(k + 1) * 128]), r(z3[k][:]),
                                     start=(k == 0), stop=(k == 3))
                o_sb = opool.tile([128, 512], f32, tag="osb")
                nc.vector.tensor_tensor(o_sb[:], zp[:], xt[:, t0:t0 + 512].bitcast(f32), AL.add)
                nc.sync.dma_start(out_d[:, t0:t0 + 512], o_sb[:])

    nc.finalize()
    return nc


# ---------------------------------------------------------------- PJRT runner

def _make_runner(nc, n_cores):
    import jax
    from jax.sharding import Mesh, PartitionSpec
    from jax.experimental.shard_map import shard_map
    import concourse.mybir as mybir
    from concourse import bass2jax

    bass2jax.install_neuronx_cc_hook()
    assert nc.is_finalized()
    partition_name = nc.partition_id_tensor.name if nc.partition_id_tensor else None

    in_names, out_names, out_avals, zero_shapes = [], [], [], []
    for alloc in nc.m.functions[0].allocations:
        if not isinstance(alloc, mybir.MemoryLocationSet):
            continue
        name = alloc.memorylocations[0].name
        if alloc.kind == "ExternalInput":
            if name != partition_name:
                in_names.append(name)
        elif alloc.kind == "ExternalOutput":
            shape = tuple(alloc.tensor_shape)
            dtype = mybir.dt.np(alloc.dtype)
            out_names.append(name)
            out_avals.append(jax.core.ShapedArray(shape, dtype))
            zero_shapes.append((shape, dtype))
    n_params = len(in_names)
    n_outs = len(out_avals)
    all_in_names = list(in_names) + list(out_names)
    if partition_name is not None:
        all_in_names.append(partition_name)
    donate = tuple(range(n_params, n_params + n_outs))

    def _body(*args):
        operands = list(args)
        if partition_name is not None:
            operands.append(bass2jax.partition_id_tensor())
        outs = bass2jax._bass_exec_p.bind(
            *operands,
            out_avals=tuple(out_avals),
            in_names=tuple(all_in_names),
            out_names=tuple(out_names),
            lowering_input_output_aliases=(),
            sim_require_finite=True,
            sim_require_nnan=True,
            nc=nc,
        )
        return tuple(outs)

    devices = jax.devices()[:n_cores]
    if n_cores == 1:
        fn = jax.jit(_body, donate_argnums=donate, keep_unused=True)
    else:
        mesh = Mesh(np.asarray(devices), ("core",))
        fn = jax.jit(
            shard_map(_body, mesh=mesh,
                      in_specs=(PartitionSpec("core"),) * (n_params + n_outs),
                      out_specs=(PartitionSpec("core"),) * n_outs,
                      check_rep=False),
            donate_argnums=donate, keep_unused=True,
        )

    def run(per_core_inputs):
        if n_cores == 1:
            ins = [np.asarray(per_core_inputs[0][n]) for n in in_names]
            zeros = [np.zeros(s, d) for s, d in zero_shapes]
        else:
            ins = [np.concatenate([np.asarray(per_core_inputs[c][n])
                                   for c in range(n_cores)], axis=0) for n in in_names]
            zeros = [np.zeros((n_cores * s[0], *s[1:]), d) for s, d in zero_shapes]
        out_arrs = fn(*ins, *zeros)
        if n_cores == 1:
            return [{name: np.asarray(out_arrs[i]) for i, name in enumerate(out_names)}]
        res = []
        for c in range(n_cores):
            d = {}
            for i, name in enumerate(out_names):
                full = np.asarray(out_arrs[i])
                d[name] = full.reshape(n_cores, *out_avals[i].shape)[c]
            res.append(d)
        return res

    run.fn = fn
    run.in_names = in_names
    run.out_names = out_names
    run.zero_shapes = zero_shapes
    return run


_RUNNER = None


def _get_runner():
    global _RUNNER
    if _RUNNER is None:
        nc = _build_program(T)
        _RUNNER = _make_runner(nc, NCORES)
    return _RUNNER


def kernel(**inputs):
    p = {k: np.asarray(v) for k, v in inputs.items()}
    consts = _host_prep(p)
    x = p['x'].astype(np.float32)            # [B, T, D]
    run = _get_runner()
    per_core = []
    for b in range(B):
        m = dict(consts)
        m['xt'] = np.ascontiguousarray(x[b].T)
        per_core.append(m)
    res = run(per_core)
    out = np.stack([res[b]['outT'].T for b in range(B)], axis=0)
    return out.astype(np.float32)
